# revision 3
# baseline (speedup 1.0000x reference)
"""Fused 2-layer peephole ConvLSTM for TRN2 (Bass/Tile), 8-core SPMD.

Problem: x[B=16, T=25, CIN=3, W=4096] -> y[B, T, HID=32, W]; two stacked
ConvLSTM layers (k=3 SAME conv over W, peephole connections), zero-init
states, scanned over T.  Data-parallel over batch: 2 batches/core,
weights replicated, no collectives.

Layout: everything quarter-packed [128 partitions = 4 quarters
(batch, w-half) x 32 channels, free = 2048 cols].  Gate pre-activations
are computed by BLOCK-DIAGONAL lhsT matmuls: a [128,128] lhsT whose
32x32 diagonal blocks hold the same per-gate weights computes that gate
for all 4 quarters in ONE [128,512] matmul (the matmul cost is
out-free-size only, and the 4x32 useful K-rows make it K-dense).  This
keeps the PE stream at its cycle floor AND every ACT/DVE op at full
128-partition density -- the previous kernel's per-(gate,quarter) ACT
passes ran at 32 partitions (4x the cost) and dominated the timeline.

Per chunk (512 cols) per layer: psA psum [128,1536] = banks i|f|g, psO
[128,512] = bank o, held open until the peephole-o matmul closes it (so
sigmoid(o) reads conv+peephole directly from PSUM, no separate add).
L0: 19 matmuls/chunk (4 x-im2col + 12 h-taps + 2 peep-if + 1 peep-o),
L1: 27 (12 h1-taps + 12 y-taps + 2 peep-if + 1 peep-o).  t=0 skips all
state matmuls (zero init).  2885us -> 1015us (TimelineSim) vs the
per-quarter-matmul version; rel err 4.1e-3.

Scheduling (the recurrence tail must never stall the PE):
  - trip = one (t, layer, chunk): [psA matmuls] [deferred peephole-o of
    the previous trip + tanh(c')] [psO matmuls] [gate ACTs + DVE c'
    chain] [s_o + h' of the previous trip].  The one-trip deferral gives
    the c'/cb' chain a full trip of slack before peephole-o needs cb'.
  - bank order g,i,f + ACT order t_g,s_i,s_f: the c' chain (DVE m2, m1,
    add, bf16 copy) starts after only two ACTs.
  - s_o is emitted after the next trip's gate ACTs so its PE dependency
    never head-of-line-blocks the ACT FIFO; halo cols + y-store DMA ride
    the last chunk's tail.
  - x is DMA-prefetched in 5-step im2col groups (rows 32q+3s+c hold
    x[b,t,c,w+d-1]; unused rows zeroed once so the block-diag x matmul
    reads finite values); bf16 datapath with fp32 cell state c.
"""

import sys

for _p in ("/opt/trn_rl_repo",):
    if _p not in sys.path:
        sys.path.insert(0, _p)

from contextlib import ExitStack

import numpy as np

import concourse.bass as bass  # noqa: F401
import concourse.tile as tile
from concourse import bacc, mybir
from concourse.bass_utils import run_bass_kernel_spmd

F32 = mybir.dt.float32
BF16 = mybir.dt.bfloat16
AF = mybir.ActivationFunctionType

import ml_dtypes

BF = ml_dtypes.bfloat16

B, T, CIN, HID, W = 16, 25, 3, 32, 4096
KTAP = 3
N_CORES = 8
B_SHARD = B // N_CORES  # 2
NQ = 4  # quarters: (batch 0/1) x (w-half 0/1); q = 2*b + half
PACK = 2048  # packed free dim (W / 2 halves)
MMN = 512  # matmul free chunk (one PSUM bank)
NCH = PACK // MMN  # 4
G = 5  # x-prefetch group size (T = 5 groups of 5)

# gate bank order i|f|g|o ; reference conv channel order is [i, f, o, g]
BANK2REF = [0, 1, 3, 2]  # bank g -> ref gate index (i,f,o,g)

# weight block index map (lhsT [128,128] blocks inside wts [128, NBLK*128])
# L0: X0[g]=g (4) ; H0[g][d]=4+3g+d (12) ; PI0=16 PF0=17 PO0=18
# L1: Y1[g][d]=19+3g+d (12) ; H1[g][d]=31+3g+d (12) ; PI1=43 PF1=44 PO1=45
NBLK = 46


def _pack_weights(conv_w0, conv_b0, wci0, wcf0, wco0, conv_w1, conv_b1, wci1, wcf1, wco1):
    conv_w0 = np.asarray(conv_w0, np.float32)
    conv_w1 = np.asarray(conv_w1, np.float32)
    blocks = np.zeros((NBLK, 128, 128), np.float32)

    def setblk(idx, q, rows, cols, mat):
        blocks[idx, 32 * q + rows[0] : 32 * q + rows[1], 32 * q + cols[0] : 32 * q + cols[1]] = mat

    for g in range(4):
        rg = BANK2REF[g]
        och = slice(32 * rg, 32 * rg + 32)
        for q in range(NQ):
            half = q & 1
            for d in range(KTAP):
                s = d if half == 0 else 2 - d
                # X0[g]: rows 3s+c, cols co = conv_w0[och, c, d]
                blocks[g, 32 * q + 3 * s : 32 * q + 3 * s + CIN, 32 * q : 32 * q + 32] = conv_w0[och, 0:CIN, d].T
                # H0[g][d]
                setblk(4 + 3 * g + d, q, (0, 32), (0, 32), conv_w0[och, CIN : CIN + HID, d].T)
                # Y1 / H1
                setblk(19 + 3 * g + d, q, (0, 32), (0, 32), conv_w1[och, 0:HID, d].T)
                setblk(31 + 3 * g + d, q, (0, 32), (0, 32), conv_w1[och, HID : 2 * HID, d].T)
    for q in range(NQ):
        setblk(16, q, (0, 32), (0, 32), np.asarray(wci0, np.float32).T)
        setblk(17, q, (0, 32), (0, 32), np.asarray(wcf0, np.float32).T)
        setblk(18, q, (0, 32), (0, 32), np.asarray(wco0, np.float32).T)
        setblk(43, q, (0, 32), (0, 32), np.asarray(wci1, np.float32).T)
        setblk(44, q, (0, 32), (0, 32), np.asarray(wcf1, np.float32).T)
        setblk(45, q, (0, 32), (0, 32), np.asarray(wco1, np.float32).T)

    wts = blocks.transpose(1, 0, 2).reshape(128, NBLK * 128).astype(BF)

    bias = np.zeros((128, 8), np.float32)
    b0 = np.asarray(conv_b0, np.float32)
    b1 = np.asarray(conv_b1, np.float32)
    for g in range(4):
        rg = BANK2REF[g]
        bias[:, g] = np.tile(b0[32 * rg : 32 * rg + 32], NQ)
        bias[:, 4 + g] = np.tile(b1[32 * rg : 32 * rg + 32], NQ)
    return dict(wts=np.ascontiguousarray(wts), bias=np.ascontiguousarray(bias))


def _build_kernel():
    nc = bacc.Bacc("TRN2", target_bir_lowering=False, debug=False)

    x_d = nc.dram_tensor("x", [B_SHARD, T, CIN, W], BF16, kind="ExternalInput")
    y_d = nc.dram_tensor("y", [B_SHARD, T, HID, W], BF16, kind="ExternalOutput")
    wts_d = nc.dram_tensor("wts", [128, NBLK * 128], BF16, kind="ExternalInput")
    bias_d = nc.dram_tensor("bias", [128, 8], F32, kind="ExternalInput")
    x_ap = x_d.ap()
    y_ap = y_d.ap()

    with tile.TileContext(nc) as tc, ExitStack() as ctx:
        const = ctx.enter_context(tc.tile_pool(name="const", bufs=1))
        xp = ctx.enter_context(tc.tile_pool(name="xp", bufs=1))
        st = ctx.enter_context(tc.tile_pool(name="st", bufs=1))
        gp = ctx.enter_context(tc.tile_pool(name="gp", bufs=2))
        psa = ctx.enter_context(tc.tile_pool(name="psa", bufs=2, space="PSUM"))
        pso = ctx.enter_context(tc.tile_pool(name="pso", bufs=2, space="PSUM"))

        # L0's blocks (0..18) first so t=0 isn't stuck behind the full 1.5MB
        NL0 = 19 * 128
        wts = const.tile([128, NBLK * 128], BF16, name="wts", tag="wts")
        nc.sync.dma_start(wts[:, 0:NL0], wts_d.ap()[:, 0:NL0])
        bias = const.tile([128, 8], F32, name="bias", tag="bias")
        nc.sync.dma_start(bias[:], bias_d.ap()[:, :])

        def WB(k):
            return wts[:, 128 * k : 128 * k + 128]

        # persistent state (ping-pong): h with 1 halo col/side
        h0 = [st.tile([128, PACK + 2], BF16, tag=f"h0_{i}", name=f"h0_{i}") for i in range(2)]
        h1 = [st.tile([128, PACK + 2], BF16, tag=f"h1_{i}", name=f"h1_{i}") for i in range(2)]
        c0 = [st.tile([128, PACK], F32, tag=f"c0_{i}", name=f"c0_{i}") for i in range(2)]
        c1 = [st.tile([128, PACK], F32, tag=f"c1_{i}", name=f"c1_{i}") for i in range(2)]
        cb0 = [st.tile([128, PACK], BF16, tag=f"cb0_{i}", name=f"cb0_{i}") for i in range(2)]
        cb1 = [st.tile([128, PACK], BF16, tag=f"cb1_{i}", name=f"cb1_{i}") for i in range(2)]

        # x im2col group tiles [128, G, PACK]; rows 32q+3s+c = x[b,t,c,w+d-1];
        # rows 32q+9..31 must be finite for the block-diag x matmul -> zero all
        # (f32 bitcast halves the memset row count; 2nd buffer on GPSIMD)
        xts = [xp.tile([128, G, PACK], BF16, tag=f"xt{i}", name=f"xt{i}") for i in range(2)]
        # split each zeroing across DVE+GPSIMD so buffer 0 is ready in ~2us
        H_G = G // 2 + 1
        nc.gpsimd.memset(xts[0][:, 0:H_G].bitcast(F32), 0.0)
        nc.vector.memset(xts[0][:, H_G:G].bitcast(F32), 0.0)
        nc.gpsimd.memset(xts[1][:, 0:H_G].bitcast(F32), 0.0)
        nc.vector.memset(xts[1][:, H_G:G].bitcast(F32), 0.0)

        # batch-edge halo cols are SAME-pad zeros, set once (inner halos are
        # maintained by halo_fix; h data cols are fully written every step)
        for h in (*h0, *h1):
            nc.vector.memset(h[0:32, 0:1], 0.0)
            nc.vector.memset(h[64:96, 0:1], 0.0)
            nc.vector.memset(h[32:64, PACK + 1 : PACK + 2], 0.0)
            nc.vector.memset(h[96:128, PACK + 1 : PACK + 2], 0.0)

        def x_group_load(gi):
            t0 = gi * G
            xt = xts[gi % 2]
            for q in range(NQ):
                b, half = q >> 1, q & 1
                w0 = half * PACK
                # group 0 gates the whole pipeline: split its issue across two
                # otherwise-idle DGE queues to halve serial issue latency
                eng = nc.sync if gi > 0 else (nc.sync if q < 2 else nc.scalar)
                for d in range(KTAP):
                    s = d if half == 0 else 2 - d
                    lo = w0 + d - 1
                    hi = lo + PACK
                    slo, shi = max(lo, 0), min(hi, W)
                    dlo = slo - lo
                    dhi = PACK - (hi - shi)
                    rows = slice(32 * q + 3 * s, 32 * q + 3 * s + CIN)
                    eng.dma_start(
                        xt[rows, 0:G, dlo:dhi],
                        x_ap[b, t0 : t0 + G, 0:CIN, slo:shi].transpose([1, 0, 2]),
                    )
                # edge tap at slot s=0 -> 32-aligned memset base
                erows = slice(32 * q, 32 * q + CIN)
                if half == 0:
                    nc.vector.memset(xt[erows, 0:G, 0:1], 0.0)
                else:
                    nc.vector.memset(xt[erows, 0:G, PACK - 1 : PACK], 0.0)

        x_group_load(0)
        nc.sync.dma_start(wts[:, NL0:], wts_d.ap()[:, NL0 : NBLK * 128])

        def halo_fix(h):
            nc.vector.tensor_copy(h[32:64, 0:1], h[0:32, PACK : PACK + 1])
            nc.vector.tensor_copy(h[0:32, PACK + 1 : PACK + 2], h[32:64, 1:2])
            nc.vector.tensor_copy(h[96:128, 0:1], h[64:96, PACK : PACK + 1])
            nc.vector.tensor_copy(h[64:96, PACK + 1 : PACK + 2], h[96:128, 1:2])

        # deferred part2 state, emitted one chunk-trip late between the next
        # trip's psA and psO matmul groups (so the peephole-o never stalls PE)
        pending = [None]

        def bank_mms(t, lyr, ch, g, xt_tl, h_prev, cb_prev, y_src):
            """(weight idx, rhs AP) list for gate bank g of chunk ch."""
            cs = slice(MMN * ch, MMN * ch + MMN)
            first_t = t == 0
            mms = []
            if lyr == 0:
                xt, tl = xt_tl
                mms.append((g, xt[:, tl, cs]))
                if not first_t:
                    for d in range(KTAP):
                        mms.append((4 + 3 * g + d, h_prev[:, MMN * ch + d : MMN * ch + d + MMN]))
            else:
                if not first_t:
                    for d in range(KTAP):
                        mms.append((31 + 3 * g + d, h_prev[:, MMN * ch + d : MMN * ch + d + MMN]))
                for d in range(KTAP):
                    mms.append((19 + 3 * g + d, y_src[:, MMN * ch + d : MMN * ch + d + MMN]))
            if g in (0, 1) and not first_t:
                mms.append(((16 if lyr == 0 else 43) + g, cb_prev[:, cs]))
            return mms

        def emit_psa(t, lyr, ch, xt_tl, h_prev, cb_prev, y_src):
            pA = psa.tile([128, 3 * MMN], F32, tag="PA", name="pA")
            for g in (2, 0, 1):  # bank g first: its ACT gates the c' chain
                out = pA[:, MMN * g : MMN * g + MMN]
                mms = bank_mms(t, lyr, ch, g, xt_tl, h_prev, cb_prev, y_src)
                for k, (wk, rhs) in enumerate(mms):
                    nc.tensor.matmul(out, WB(wk), rhs, start=(k == 0),
                                     stop=(k == len(mms) - 1))
            return pA

        def emit_pso(t, lyr, ch, xt_tl, h_prev, cb_prev, y_src):
            pO = pso.tile([128, MMN], F32, tag="PO", name="pO")
            mms = bank_mms(t, lyr, ch, 3, xt_tl, h_prev, cb_prev, y_src)
            for k, (wk, rhs) in enumerate(mms):
                # group stays open; closed by the peephole-o matmul in part2
                nc.tensor.matmul(pO[:, 0:MMN], WB(wk), rhs, start=(k == 0), stop=False)
            return pO

        def emit_part1(t, lyr, ch, pA, sif, c_prev, c_nxt, cb_nxt, bb):
            cs = slice(MMN * ch, MMN * ch + MMN)
            first_t = t == 0
            # ACT order t_g, s_i, s_f matches bank close order (g, i, f);
            # the DVE c' chain starts after only two ACTs
            nc.scalar.activation(sif[:, 2 * PACK + MMN * ch : 2 * PACK + MMN * ch + MMN],
                                 pA[:, 2 * MMN : 3 * MMN], AF.Tanh, bias=bias[:, bb + 2 : bb + 3])
            nc.scalar.activation(sif[:, 0 * PACK + MMN * ch : 0 * PACK + MMN * ch + MMN],
                                 pA[:, 0:MMN], AF.Sigmoid, bias=bias[:, bb : bb + 1])
            nc.scalar.activation(sif[:, 1 * PACK + MMN * ch : 1 * PACK + MMN * ch + MMN],
                                 pA[:, MMN : 2 * MMN], AF.Sigmoid, bias=bias[:, bb + 1 : bb + 2])
            m2 = gp.tile([128, MMN], F32, tag="m2", name="m2")
            nc.vector.tensor_mul(m2[:], sif[:, 0 * PACK + MMN * ch : 0 * PACK + MMN * ch + MMN],
                                 sif[:, 2 * PACK + MMN * ch : 2 * PACK + MMN * ch + MMN])
            if first_t:
                nc.vector.tensor_copy(c_nxt[:, cs], m2[:])
            else:
                m1 = gp.tile([128, MMN], F32, tag="m1", name="m1")
                nc.vector.tensor_mul(m1[:], sif[:, 1 * PACK + MMN * ch : 1 * PACK + MMN * ch + MMN],
                                     c_prev[:, cs])
                nc.vector.tensor_add(c_nxt[:, cs], m1[:], m2[:])
            nc.vector.tensor_copy(cb_nxt[:, cs], c_nxt[:, cs])

        def emit_part2a(p2):
            # peephole-o + t_c + o-bank evacuation: the idle GPSIMD copies the
            # o-bank to SBUF right after peephole-o so the pso buffer frees
            # without waiting for ACT's queue (s_o then reads SBUF at leisure)
            t, lyr, ch = p2["t"], p2["lyr"], p2["ch"]
            cs = slice(MMN * ch, MMN * ch + MMN)
            pk = 18 if lyr == 0 else 45
            nc.tensor.matmul(p2["pO"][:, 0:MMN], WB(pk), p2["cb_nxt"][:, cs],
                             start=False, stop=True)
            nc.scalar.activation(p2["tc"][:, cs], p2["c_nxt"][:, cs], AF.Tanh)

        def emit_part2b(p2):
            # s_o is emitted after the NEXT trip's gate ACTs so its PE
            # dependency (peephole-o) is long satisfied -> no ACT FIFO stall
            t, lyr, ch, bb = p2["t"], p2["lyr"], p2["ch"], p2["bb"]
            cs = slice(MMN * ch, MMN * ch + MMN)
            h_nxt = p2["h_nxt"]
            nc.scalar.activation(p2["so"][:, cs], p2["pO"][:, 0:MMN], AF.Sigmoid,
                                 bias=bias[:, bb + 3 : bb + 4])
            nc.vector.tensor_mul(h_nxt[:, 1 + MMN * ch : 1 + MMN * ch + MMN],
                                 p2["so"][:, cs], p2["tc"][:, cs])
            if p2["last"]:
                halo_fix(h_nxt)
                if lyr == 1:
                    wend = PACK if t < T - 1 else MMN
                    for q in range(NQ):
                        b, half = q >> 1, q & 1
                        nc.sync.dma_start(
                            y_ap[b, t, 0:HID, half * PACK : half * PACK + wend],
                            h_nxt[32 * q : 32 * q + 32, 1 : wend + 1],
                        )

        # trip sequence: sequential layers; halo-dependent chunk 0 last in L1
        # (measured: interleaving L0(t+1) trips into L1(t) regresses ~20us)
        L0_ORDER = (0, 1, 2, 3)
        L1_ORDER = (1, 2, 3, 0)
        seq = []
        for t in range(T):
            for k in range(NCH):
                seq.append((t, 0, L0_ORDER[k], k == NCH - 1))
            for k in range(NCH):
                seq.append((t, 1, L1_ORDER[k], k == NCH - 1))

        # per-(t,lyr) scratch tiles, allocated lazily at first trip
        scratch = {}

        def get_scratch(t, lyr):
            key = (t, lyr)
            if key not in scratch:
                scratch[key] = (
                    gp.tile([128, 3 * PACK], F32, tag="sif", name="sif"),
                    gp.tile([128, PACK], F32, tag="so", name="so"),
                    gp.tile([128, PACK], F32, tag="tc", name="tc"),
                )
                scratch.pop((t - 2, lyr), None)
            return scratch[key]

        for t, lyr, ch, last in seq:
            cur, nxt = t % 2, (t + 1) % 2
            gi, tl = t // G, t % G
            if lyr == 0 and ch == 0 and tl == 0 and gi + 1 < (T + G - 1) // G:
                x_group_load(gi + 1)
            if lyr == 0:
                h_prev, cb_prev, c_prev = h0[cur], cb0[cur], c0[cur]
                c_nxt, cb_nxt, h_nxt = c0[nxt], cb0[nxt], h0[nxt]
                y_src = None
                bb = 0
                xt_tl = (xts[gi % 2], tl)
            else:
                h_prev, cb_prev, c_prev = h1[cur], cb1[cur], c1[cur]
                c_nxt, cb_nxt, h_nxt = c1[nxt], cb1[nxt], h1[nxt]
                y_src = h0[nxt]
                bb = 4
                xt_tl = None
            sif, so_t, tc_t = get_scratch(t, lyr)
            pA = emit_psa(t, lyr, ch, xt_tl, h_prev, cb_prev, y_src)
            p = pending[0]
            if p is not None:
                if p["pO"] is None:  # L1 trips defer their psO by one trip
                    p["pO"] = emit_pso(*p["psoargs"])
                emit_part2a(p)
            pO = emit_pso(t, lyr, ch, xt_tl, h_prev, cb_prev, y_src)
            emit_part1(t, lyr, ch, pA, sif, c_prev, c_nxt, cb_nxt, bb)
            if p is not None:
                emit_part2b(p)
            pending[0] = dict(
                t=t, lyr=lyr, ch=ch, pO=pO, so=so_t, tc=tc_t, c_nxt=c_nxt,
                cb_nxt=cb_nxt, h_nxt=h_nxt, bb=bb, last=last,
                psoargs=(t, lyr, ch, xt_tl, h_prev, cb_prev, y_src),
            )
        p = pending[0]
        if p["pO"] is None:
            p["pO"] = emit_pso(*p["psoargs"])
        # final step: cols 512:2048 (chunks 1-3) are already in h1 -- stream
        # them out now so only the 512-col chunk-0 stores sit on the drain
        for q in range(NQ):
            b, half = q >> 1, q & 1
            nc.sync.dma_start(
                y_ap[b, T - 1, 0:HID, half * PACK + MMN : half * PACK + PACK],
                h1[T % 2][32 * q : 32 * q + 32, 1 + MMN : PACK + 1],
            )
        emit_part2a(p)
        emit_part2b(p)

    nc.compile()
    return nc


_NC_CACHE = None


def _get_nc():
    global _NC_CACHE
    if _NC_CACHE is None:
        _NC_CACHE = _build_kernel()
    return _NC_CACHE


def kernel(x, conv_w0, conv_b0, wci0, wcf0, wco0,
           conv_w1, conv_b1, wci1, wcf1, wco1):
    x = np.ascontiguousarray(np.asarray(x, np.float32).astype(BF))
    packed = _pack_weights(conv_w0, conv_b0, wci0, wcf0, wco0,
                           conv_w1, conv_b1, wci1, wcf1, wco1)
    nc = _get_nc()
    in_maps = []
    for core in range(N_CORES):
        m = {"x": np.ascontiguousarray(x[B_SHARD * core : B_SHARD * (core + 1)])}
        m.update(packed)
        in_maps.append(m)
    res = run_bass_kernel_spmd(nc, in_maps, core_ids=list(range(N_CORES)))
    return np.concatenate(
        [np.asarray(r["y"]).astype(np.float32) for r in res.results], axis=0)
